# revision 18
# baseline (speedup 1.0000x reference)
"""Trainium2 Bass kernel for nn_NaiveE2V (gnn_message_passing).

Math (reference):
    w0 = W[0][orders]; w1 = W[1][orders]                        # [e,d,d] gathers
    x0 = concat(x_v @ W[0,1], einsum('ei,eij->ej', x_e, w0)).mean(0)   # [1,d]
    x1 = (x_v @ W[1,1] + incidence @ einsum(x_e, w1)) / (1+sn[:,None])
    out = x0 + x1 + b                                            # [n,d]

Kernel strategy (8 cores, vertex-sharded, no collectives):
  * Heavy traffic is `incidence` (4000 x 16000 fp32 = 256 MB). Each core
    owns 500 vertices = 500 columns of incidence.T -> 32 MB per core,
    read exactly once (memory roofline). In f16 mode the incidence and
    x_e streams are sent as fp16 (halved traffic; fp16 keeps 11 mantissa
    bits and the incidence values live in [0,1), so the rounding noise
    stays ~1e-4 of the output scale; the contraction accumulates in fp32
    PSUM either way).
  * Host prep (cheap O(N*E) passes, no flops): sort edges by order, pad
    each order group to a multiple of 128, and interleave edges within
    each group (position (j, p) <- sorted offset p*tiles_k + j) so that
    each 128-edge matmul tile stays order-pure while the incidence.T DMA
    descriptors become long contiguous runs per partition. Fold the
    1/(1+suffix_normalizer) row scaling into incidence and x_v; use
    [d, *] layouts so (x0 + b) is a per-partition scalar. Padded x_e
    rows are zeroed, so padded incidence.T rows can hold garbage (their
    rank-1 term is 0 @ row = 0). The contraction over edges is
    permutation-invariant, so any consistent edge order works.
  * x0 needs only per-order sums of x_e and the x_v sum (host-reduced
    [64, 6] input) fed through tiny [64,1] matmuls on device.
  * DMA: const loads ride the scalar HWDGE ring, the incidence stream
    rides the sync ring, so streaming is not FIFO-blocked behind consts.
  * On device (per core):
      x1_e tile (natural [128e, 64]) = xet_tile.T @ W1[k]        (PE)
      aggT [64, 500] += x1_e_t.T @ incT_tile                     (PE, PSUM accum)
      aggT += W11.T @ xvrt   (full-precision x1_v term)          (PE)
      out.T = aggT + (x0 + b)                                    (DVE, per-part scalar)
  * Host: concat per-core [64,500] outputs, transpose to [4000, 64].
"""

import os
import numpy as np

N, E, D, NK = 4000, 16000, 64, 5
NCORES = 8
VS = N // NCORES            # 500 vertices per core
P = 128
SUPER = 8                   # edge tiles per DMA batch
XCHUNK_TILES = 32           # xet DMA chunk size, in tiles
INV_TOTAL = 1.0 / (N + E)

# "f16": fp16 incidence/x_e streams (half DMA, full-rate PE).
# "f32r": fp32 data with single-pass float32r matmuls (max precision).
MODE = os.environ.get("KERNEL_MODE", "f16")

# Set to "1" (env KERNEL_TRACE) before import to capture NTFF timing into
# LAST_EXEC_NS after each kernel() call.
TRACE = os.environ.get("KERNEL_TRACE", "0") == "1"
LAST_EXEC_NS = None
LAST_RESULTS = None


def _ensure_ntff_hook():
    """Register the axon NTFF profiling hook if the image's antenv lacks it."""
    try:
        from antenv.axon_hooks import get_axon_ntff_profile_hook  # noqa: F401
        return True
    except ImportError:
        pass
    try:
        import sys
        import types

        import antenv
        from trn_agent_boot.trn_boot import _ntff_profile_via_ctypes

        hook = _ntff_profile_via_ctypes("/opt/axon/libaxon_pjrt.so")
        mod = types.ModuleType("antenv.axon_hooks")
        mod.get_axon_ntff_profile_hook = lambda: hook
        mod.set_axon_ntff_profile_hook = lambda h: None
        sys.modules["antenv.axon_hooks"] = mod
        antenv.axon_hooks = mod
        return hook is not None
    except Exception:
        return False


def _build_program(group_tiles):
    """One SPMD program (identical across cores; per-core data differs).

    group_tiles: number of 128-edge tiles per order group k (len NK).
    """
    import concourse.mybir as mybir
    import concourse.tile as tile
    from concourse import bacc

    f32 = mybir.dt.float32
    f32r = mybir.dt.float32r
    fstream = mybir.dt.float16 if MODE == "f16" else f32r
    OP = mybir.AluOpType

    n_tiles = sum(group_tiles)
    e_pad = n_tiles * P
    g_start = np.concatenate([[0], np.cumsum(group_tiles)])  # in tiles
    nz = [k for k in range(NK) if group_tiles[k] > 0]

    nc = bacc.Bacc("TRN2", target_bir_lowering=False, debug=False,
                   enable_asserts=False)

    xet_d = nc.dram_tensor("xet", [D, e_pad], fstream, kind="ExternalInput")
    inct_d = nc.dram_tensor("inct", [e_pad, VS], fstream, kind="ExternalInput")
    xvrt_d = nc.dram_tensor("xvrt", [D, VS], f32r, kind="ExternalInput")
    w0_d = nc.dram_tensor("w0", [NK, D, D], f32, kind="ExternalInput")
    w1_d = nc.dram_tensor("w1", [NK, D, D], fstream, kind="ExternalInput")
    w11_d = nc.dram_tensor("w11", [D, D], f32r, kind="ExternalInput")
    bt_d = nc.dram_tensor("bt", [D, 1], f32, kind="ExternalInput")
    s6_d = nc.dram_tensor("s6", [D, NK + 1], f32, kind="ExternalInput")
    outt_d = nc.dram_tensor("outt", [D, VS], f32, kind="ExternalOutput")

    # xet chunking for startup overlap
    xchunks = []
    c0 = 0
    while c0 < n_tiles:
        c1 = min(c0 + XCHUNK_TILES, n_tiles)
        xchunks.append((c0, c1))
        c0 = c1

    inc_bufs = 10 if MODE == "f16" else 4
    with tile.TileContext(nc) as tc:
        with (
            tc.tile_pool(name="consts", bufs=1) as consts,
            tc.tile_pool(name="incp", bufs=inc_bufs) as inc_pool,
            tc.tile_pool(name="x1ep", bufs=n_tiles + 2) as x1e_pool,
            tc.tile_pool(name="pxp", bufs=4, space="PSUM") as px_pool,
            tc.tile_pool(name="paccp", bufs=1, space="PSUM") as pacc_pool,
            tc.tile_pool(name="warmp", bufs=1, space="PSUM") as warm_pool,
        ):
            # ---- PE warm-up: dense dummy matmuls while the first DMAs land.
            # The HAM throttle keeps the PE at half clock until it sees ~4us
            # of continuous matmul work; burn the DMA startup window ramping
            # so the real stream runs at full clock.
            wsb = consts.tile([P, 512], mybir.dt.float16)
            nc.vector.memset(wsb[:], 0.0)
            wps = warm_pool.tile([P, 512], f32)
            for _ in range(18):
                nc.tensor.matmul(wps[:], lhsT=wsb[:, :P], rhs=wsb[:],
                                 start=True, stop=True)
            # ---- constant loads (scalar HWDGE ring; tiny ones first so no
            # PE instruction ever stalls on them) ----
            w1 = consts.tile([D, NK, D], fstream)
            nc.scalar.dma_start(w1[:], w1_d.ap().rearrange("k i j -> i k j"))
            w0 = consts.tile([D, NK, D], f32)
            nc.scalar.dma_start(w0[:], w0_d.ap().rearrange("k i j -> i k j"))
            bt = consts.tile([D, 1], f32)
            nc.scalar.dma_start(bt[:], bt_d[:])
            s6 = consts.tile([D, NK + 1], f32)
            nc.scalar.dma_start(s6[:], s6_d[:])
            w11 = consts.tile([D, D], f32r)
            nc.scalar.dma_start(w11[:], w11_d[:])
            xvrt = consts.tile([D, VS], f32r)
            nc.scalar.dma_start(xvrt[:], xvrt_d[:])
            xet_tiles = []
            for (t0, t1) in xchunks:
                xt = consts.tile([D, (t1 - t0) * P], fstream, tag=f"xet{t0}")
                nc.scalar.dma_start(xt[:], xet_d[:, t0 * P:t1 * P])
                xet_tiles.append(xt)

            def xet_slice(t):
                ci = t // XCHUNK_TILES
                off = (t - xchunks[ci][0]) * P
                return xet_tiles[ci][:, off:off + P]

            # ---- phase 1: produce ALL x1_e tiles (PE + copy engines) while
            # the incidence stream loads in the background ----
            tile_k = []          # order group of tile t
            for k in nz:
                tile_k.extend([k] * int(group_tiles[k]))
            x1e_tiles = []
            for t in range(n_tiles):
                px = px_pool.tile([P, D], f32, tag="px")
                nc.tensor.matmul(
                    px[:], lhsT=xet_slice(t), rhs=w1[:, tile_k[t], :],
                    start=True, stop=True,
                )
                x1e = x1e_pool.tile([P, D], fstream, tag="x1e")
                if t % 2 == 0:
                    nc.vector.tensor_copy(out=x1e[:], in_=px[:])
                else:
                    nc.scalar.copy(out=x1e[:], in_=px[:])
                x1e_tiles.append(x1e)

            # ---- phase 2: pure streaming aggregation, paced by DMA ----
            # Edge order within group k is interleaved on host: matmul tile
            # (k, j) holds the edges at sorted offsets {p*tiles_k + j}, so
            # the inct DMA for chunk [j0, j0+nt) is one long contiguous run
            # per partition.
            pagg = pacc_pool.tile([D, VS], f32)
            first = True
            ci = 0
            for gi, k in enumerate(nz):
                tiles_k = int(group_tiles[k])
                row0 = int(g_start[k]) * P
                g_ap = inct_d[row0:row0 + tiles_k * P, :].rearrange(
                    "(p o) n -> p o n", p=P)
                for j0 in range(0, tiles_k, SUPER):
                    nt = min(SUPER, tiles_k - j0)
                    itile = inc_pool.tile([P, SUPER, VS], fstream, tag="inct")
                    # alternate HWDGE rings so chunk issue isn't FIFO-coupled
                    dma_eng = nc.sync if ci % 2 == 0 else nc.scalar
                    ci += 1
                    dma_eng.dma_start(itile[:, :nt, :], g_ap[:, j0:j0 + nt, :])
                    for j in range(nt):
                        t = int(g_start[k]) + j0 + j
                        nc.tensor.matmul(
                            pagg[:], lhsT=x1e_tiles[t][:], rhs=itile[:, j, :],
                            start=first, stop=False,
                        )
                        first = False

            # x1_v term folded into the same accumulation (full precision)
            nc.tensor.matmul(pagg[:], lhsT=w11[:], rhs=xvrt[:],
                             start=False, stop=True)

            # ---- x0 path (off the critical path): tiny matmuls ----
            p0 = pacc_pool.tile([D, 1], f32)
            terms = [(k, k) for k in range(NK)] + [(1, NK)]  # (w idx, s6 col)
            for i, (k, col) in enumerate(terms):
                nc.tensor.matmul(
                    p0[:], lhsT=w0[:, k, :], rhs=s6[:, col:col + 1],
                    start=(i == 0), stop=(i == len(terms) - 1),
                )
            x0b = consts.tile([D, 1], f32)
            # x0b = p0 / (N+E) + b.T   (per-partition scalar for the final add)
            nc.vector.tensor_scalar(
                out=x0b[:], in0=p0[:], scalar1=INV_TOTAL, scalar2=bt[:],
                op0=OP.mult, op1=OP.add,
            )

            outt = consts.tile([D, VS], f32)
            nc.vector.tensor_scalar(out=outt[:], in0=pagg[:], scalar1=x0b[:],
                                    scalar2=None, op0=OP.add)
            nc.sync.dma_start(outt_d[:], outt[:])

    nc.compile()
    return nc


def kernel(x_v, x_e, incidence, edge_orders, suffix_normalizer, W, b):
    global LAST_EXEC_NS, LAST_RESULTS
    from concourse.bass_utils import run_bass_kernel_spmd

    x_v = np.ascontiguousarray(np.asarray(x_v, dtype=np.float32))
    x_e = np.ascontiguousarray(np.asarray(x_e, dtype=np.float32))
    incidence = np.asarray(incidence, dtype=np.float32)
    eo = np.asarray(edge_orders).astype(np.int64)
    sn = np.asarray(suffix_normalizer, dtype=np.float32)
    W = np.asarray(W, dtype=np.float32)
    b = np.asarray(b, dtype=np.float32)

    np_stream = np.float16 if MODE == "f16" else np.float32

    # ---- host prep: sort by order, pad groups to 128, interleave in-group --
    counts = np.bincount(eo, minlength=NK)
    assert counts.size == NK, f"edge order out of range: {counts.size}"

    group_tiles = [(int(c) + P - 1) // P for c in counts]
    permA_parts = []     # A rows: padded sorted order (pad rows: garbage OK)
    permX_parts = []     # xet cols: interleaved within group
    valid_parts = []     # False where xet slot is padding
    for k in range(NK):
        idx = np.nonzero(eo == k)[0]
        tk = group_tiles[k]
        if tk == 0:
            continue
        gsz = tk * P
        src = np.zeros(gsz, dtype=np.int64)
        val = np.zeros(gsz, dtype=bool)
        src[:len(idx)] = idx
        val[:len(idx)] = True
        permA_parts.append(src)
        # interleave: final slot (j, p) (j = tile in group, p = partition)
        # takes sorted-group offset p*tk + j — matches the DMA access
        # pattern "(p o) n" that hands partition p rows p*tk + [j0, j0+nt)
        permX_parts.append(src.reshape(P, tk).T.reshape(-1))
        valid_parts.append(val.reshape(P, tk).T.reshape(-1))
    permA = np.concatenate(permA_parts)
    permX = np.concatenate(permX_parts)
    valid = np.concatenate(valid_parts)

    xe_pad = x_e[permX]
    xe_pad[~valid] = 0.0
    xet = np.ascontiguousarray(xe_pad.T.astype(np_stream))   # [64, e_pad]
    r = (1.0 / (1.0 + sn)).astype(np.float32)
    A = incidence.T[permA]                                   # [e_pad, N]
    A *= r[None, :]
    A = A.astype(np_stream)
    xvrt_full = np.ascontiguousarray((x_v * r[:, None]).T)   # [64, N]
    w0 = np.ascontiguousarray(W[0])
    w1 = np.ascontiguousarray(W[1]).astype(np_stream)
    w11 = np.ascontiguousarray(W[1, 1])
    bt = np.ascontiguousarray(b.reshape(1, D).T)             # [64, 1]

    # host-side reductions feeding the tiny x0 matmuls
    s6 = np.zeros((D, NK + 1), dtype=np.float32)
    for k in range(NK):
        if counts[k]:
            s6[:, k] = x_e[eo == k].sum(axis=0)
    s6[:, NK] = x_v.sum(axis=0)

    nc = _build_program(group_tiles)

    in_maps = []
    for m in range(NCORES):
        sl = slice(m * VS, (m + 1) * VS)
        in_maps.append({
            "xet": xet,
            "inct": np.ascontiguousarray(A[:, sl]),
            "xvrt": np.ascontiguousarray(xvrt_full[:, sl]),
            "w0": w0,
            "w1": w1,
            "w11": w11,
            "bt": bt,
            "s6": s6,
        })
    del A

    do_trace = TRACE and _ensure_ntff_hook()
    res = run_bass_kernel_spmd(nc, in_maps, core_ids=list(range(NCORES)),
                               trace=do_trace)
    LAST_EXEC_NS = res.exec_time_ns
    LAST_RESULTS = res

    out = np.empty((N, D), dtype=np.float32)
    for m in range(NCORES):
        out[m * VS:(m + 1) * VS, :] = res.results[m]["outt"].T
    return out


# revision 19
# speedup vs baseline: 1.0302x; 1.0302x over previous
"""Trainium2 Bass kernel for nn_NaiveE2V (gnn_message_passing).

Math (reference):
    w0 = W[0][orders]; w1 = W[1][orders]                        # [e,d,d] gathers
    x0 = concat(x_v @ W[0,1], einsum('ei,eij->ej', x_e, w0)).mean(0)   # [1,d]
    x1 = (x_v @ W[1,1] + incidence @ einsum(x_e, w1)) / (1+sn[:,None])
    out = x0 + x1 + b                                            # [n,d]

Kernel strategy (8 cores, vertex-sharded, no collectives):
  * Heavy traffic is `incidence` (4000 x 16000 fp32 = 256 MB). Each core
    owns 500 vertices = 500 columns of incidence.T -> 32 MB per core,
    read exactly once (memory roofline). In f16 mode the incidence and
    x_e streams are sent as fp16 (halved traffic; fp16 keeps 11 mantissa
    bits and the incidence values live in [0,1), so the rounding noise
    stays ~1e-4 of the output scale; the contraction accumulates in fp32
    PSUM either way).
  * Host prep (cheap O(N*E) passes, no flops): sort edges by order, pad
    each order group to a multiple of 128, and interleave edges within
    each group (position (j, p) <- sorted offset p*tiles_k + j) so that
    each 128-edge matmul tile stays order-pure while the incidence.T DMA
    descriptors become long contiguous runs per partition. Fold the
    1/(1+suffix_normalizer) row scaling into incidence and x_v; use
    [d, *] layouts so (x0 + b) is a per-partition scalar. Padded x_e
    rows are zeroed, so padded incidence.T rows can hold garbage (their
    rank-1 term is 0 @ row = 0). The contraction over edges is
    permutation-invariant, so any consistent edge order works.
  * x0 needs only per-order sums of x_e and the x_v sum (host-reduced
    [64, 6] input) fed through tiny [64,1] matmuls on device.
  * DMA: const loads ride the scalar HWDGE ring, the incidence stream
    rides the sync ring, so streaming is not FIFO-blocked behind consts.
  * On device (per core):
      x1_e tile (natural [128e, 64]) = xet_tile.T @ W1[k]        (PE)
      aggT [64, 500] += x1_e_t.T @ incT_tile                     (PE, PSUM accum)
      aggT += W11.T @ xvrt   (full-precision x1_v term)          (PE)
      out.T = aggT + (x0 + b)                                    (DVE, per-part scalar)
  * Host: concat per-core [64,500] outputs, transpose to [4000, 64].
"""

import os
import numpy as np

N, E, D, NK = 4000, 16000, 64, 5
NCORES = 8
VS = N // NCORES            # 500 vertices per core
P = 128
SUPER = 16                  # edge tiles per DMA batch
XCHUNK_TILES = 32           # xet DMA chunk size, in tiles
INV_TOTAL = 1.0 / (N + E)

# "f16": fp16 incidence/x_e streams (half DMA, full-rate PE).
# "f32r": fp32 data with single-pass float32r matmuls (max precision).
MODE = os.environ.get("KERNEL_MODE", "f16")

# Set to "1" (env KERNEL_TRACE) before import to capture NTFF timing into
# LAST_EXEC_NS after each kernel() call.
TRACE = os.environ.get("KERNEL_TRACE", "0") == "1"
LAST_EXEC_NS = None
LAST_RESULTS = None


def _ensure_ntff_hook():
    """Register the axon NTFF profiling hook if the image's antenv lacks it."""
    try:
        from antenv.axon_hooks import get_axon_ntff_profile_hook  # noqa: F401
        return True
    except ImportError:
        pass
    try:
        import sys
        import types

        import antenv
        from trn_agent_boot.trn_boot import _ntff_profile_via_ctypes

        hook = _ntff_profile_via_ctypes("/opt/axon/libaxon_pjrt.so")
        mod = types.ModuleType("antenv.axon_hooks")
        mod.get_axon_ntff_profile_hook = lambda: hook
        mod.set_axon_ntff_profile_hook = lambda h: None
        sys.modules["antenv.axon_hooks"] = mod
        antenv.axon_hooks = mod
        return hook is not None
    except Exception:
        return False


def _build_program(group_tiles):
    """One SPMD program (identical across cores; per-core data differs).

    group_tiles: number of 128-edge tiles per order group k (len NK).
    """
    import concourse.mybir as mybir
    import concourse.tile as tile
    from concourse import bacc

    f32 = mybir.dt.float32
    f32r = mybir.dt.float32r
    fstream = mybir.dt.float16 if MODE == "f16" else f32r
    OP = mybir.AluOpType

    n_tiles = sum(group_tiles)
    e_pad = n_tiles * P
    g_start = np.concatenate([[0], np.cumsum(group_tiles)])  # in tiles
    nz = [k for k in range(NK) if group_tiles[k] > 0]

    nc = bacc.Bacc("TRN2", target_bir_lowering=False, debug=False,
                   enable_asserts=False)

    xet_d = nc.dram_tensor("xet", [D, e_pad], fstream, kind="ExternalInput")
    inct_d = nc.dram_tensor("inct", [e_pad, VS], fstream, kind="ExternalInput")
    xvrt_d = nc.dram_tensor("xvrt", [D, VS], f32r, kind="ExternalInput")
    w0_d = nc.dram_tensor("w0", [NK, D, D], f32, kind="ExternalInput")
    w1_d = nc.dram_tensor("w1", [NK, D, D], fstream, kind="ExternalInput")
    w11_d = nc.dram_tensor("w11", [D, D], f32r, kind="ExternalInput")
    bt_d = nc.dram_tensor("bt", [D, 1], f32, kind="ExternalInput")
    s6_d = nc.dram_tensor("s6", [D, NK + 1], f32, kind="ExternalInput")
    outt_d = nc.dram_tensor("outt", [D, VS], f32, kind="ExternalOutput")

    # xet chunking for startup overlap
    xchunks = []
    c0 = 0
    while c0 < n_tiles:
        c1 = min(c0 + XCHUNK_TILES, n_tiles)
        xchunks.append((c0, c1))
        c0 = c1

    inc_bufs = 5 if MODE == "f16" else 4
    with tile.TileContext(nc) as tc:
        with (
            tc.tile_pool(name="consts", bufs=1) as consts,
            tc.tile_pool(name="incp", bufs=inc_bufs) as inc_pool,
            tc.tile_pool(name="x1ep", bufs=n_tiles + 2) as x1e_pool,
            tc.tile_pool(name="pxp", bufs=4, space="PSUM") as px_pool,
            tc.tile_pool(name="paccp", bufs=1, space="PSUM") as pacc_pool,
            tc.tile_pool(name="warmp", bufs=1, space="PSUM") as warm_pool,
        ):
            # ---- PE warm-up: dense dummy matmuls while the first DMAs land.
            # The HAM throttle keeps the PE at half clock until it sees ~4us
            # of continuous matmul work; burn the DMA startup window ramping
            # so the real stream runs at full clock.
            wsb = consts.tile([P, 512], mybir.dt.float16)
            nc.vector.memset(wsb[:], 0.0)
            wps = warm_pool.tile([P, 512], f32)
            for _ in range(14):
                nc.tensor.matmul(wps[:], lhsT=wsb[:, :P], rhs=wsb[:],
                                 start=True, stop=True)
            # ---- constant loads (scalar HWDGE ring; tiny ones first so no
            # PE instruction ever stalls on them) ----
            w1 = consts.tile([D, NK, D], fstream)
            nc.scalar.dma_start(w1[:], w1_d.ap().rearrange("k i j -> i k j"))
            xet_tiles = []
            for (t0, t1) in xchunks:
                xt = consts.tile([D, (t1 - t0) * P], fstream, tag=f"xet{t0}")
                nc.scalar.dma_start(xt[:], xet_d[:, t0 * P:t1 * P])
                xet_tiles.append(xt)
            w0 = consts.tile([D, NK, D], f32)
            nc.scalar.dma_start(w0[:], w0_d.ap().rearrange("k i j -> i k j"))
            bt = consts.tile([D, 1], f32)
            nc.scalar.dma_start(bt[:], bt_d[:])
            s6 = consts.tile([D, NK + 1], f32)
            nc.scalar.dma_start(s6[:], s6_d[:])
            w11 = consts.tile([D, D], f32r)
            nc.scalar.dma_start(w11[:], w11_d[:])
            xvrt = consts.tile([D, VS], f32r)
            nc.scalar.dma_start(xvrt[:], xvrt_d[:])

            def xet_slice(t):
                ci = t // XCHUNK_TILES
                off = (t - xchunks[ci][0]) * P
                return xet_tiles[ci][:, off:off + P]

            # ---- phase 1: produce ALL x1_e tiles (PE + copy engines) while
            # the incidence stream loads in the background ----
            tile_k = []          # order group of tile t
            for k in nz:
                tile_k.extend([k] * int(group_tiles[k]))
            x1e_tiles = []
            for t in range(n_tiles):
                px = px_pool.tile([P, D], f32, tag="px")
                nc.tensor.matmul(
                    px[:], lhsT=xet_slice(t), rhs=w1[:, tile_k[t], :],
                    start=True, stop=True,
                )
                x1e = x1e_pool.tile([P, D], fstream, tag="x1e")
                if t % 2 == 0:
                    nc.vector.tensor_copy(out=x1e[:], in_=px[:])
                else:
                    nc.scalar.copy(out=x1e[:], in_=px[:])
                x1e_tiles.append(x1e)

            # ---- phase 2: pure streaming aggregation, paced by DMA ----
            # Edge order within group k is interleaved on host: matmul tile
            # (k, j) holds the edges at sorted offsets {p*tiles_k + j}, so
            # the inct DMA for chunk [j0, j0+nt) is one long contiguous run
            # per partition.
            pagg = pacc_pool.tile([D, VS], f32)
            first = True
            ci = 0
            for gi, k in enumerate(nz):
                tiles_k = int(group_tiles[k])
                row0 = int(g_start[k]) * P
                g_ap = inct_d[row0:row0 + tiles_k * P, :].rearrange(
                    "(p o) n -> p o n", p=P)
                for j0 in range(0, tiles_k, SUPER):
                    nt = min(SUPER, tiles_k - j0)
                    itile = inc_pool.tile([P, SUPER, VS], fstream, tag="inct")
                    # alternate HWDGE rings so chunk issue isn't FIFO-coupled
                    dma_eng = nc.sync if ci % 2 == 0 else nc.scalar
                    ci += 1
                    dma_eng.dma_start(itile[:, :nt, :], g_ap[:, j0:j0 + nt, :])
                    for j in range(nt):
                        t = int(g_start[k]) + j0 + j
                        nc.tensor.matmul(
                            pagg[:], lhsT=x1e_tiles[t][:], rhs=itile[:, j, :],
                            start=first, stop=False,
                        )
                        first = False

            # x1_v term folded into the same accumulation (full precision)
            nc.tensor.matmul(pagg[:], lhsT=w11[:], rhs=xvrt[:],
                             start=False, stop=True)

            # ---- x0 path (off the critical path): tiny matmuls ----
            p0 = pacc_pool.tile([D, 1], f32)
            terms = [(k, k) for k in range(NK)] + [(1, NK)]  # (w idx, s6 col)
            for i, (k, col) in enumerate(terms):
                nc.tensor.matmul(
                    p0[:], lhsT=w0[:, k, :], rhs=s6[:, col:col + 1],
                    start=(i == 0), stop=(i == len(terms) - 1),
                )
            x0b = consts.tile([D, 1], f32)
            # x0b = p0 / (N+E) + b.T   (per-partition scalar for the final add)
            nc.vector.tensor_scalar(
                out=x0b[:], in0=p0[:], scalar1=INV_TOTAL, scalar2=bt[:],
                op0=OP.mult, op1=OP.add,
            )

            outt = consts.tile([D, VS], f32)
            nc.vector.tensor_scalar(out=outt[:], in0=pagg[:], scalar1=x0b[:],
                                    scalar2=None, op0=OP.add)
            nc.sync.dma_start(outt_d[:], outt[:])

    nc.compile()
    return nc


def kernel(x_v, x_e, incidence, edge_orders, suffix_normalizer, W, b):
    global LAST_EXEC_NS, LAST_RESULTS
    from concourse.bass_utils import run_bass_kernel_spmd

    x_v = np.ascontiguousarray(np.asarray(x_v, dtype=np.float32))
    x_e = np.ascontiguousarray(np.asarray(x_e, dtype=np.float32))
    incidence = np.asarray(incidence, dtype=np.float32)
    eo = np.asarray(edge_orders).astype(np.int64)
    sn = np.asarray(suffix_normalizer, dtype=np.float32)
    W = np.asarray(W, dtype=np.float32)
    b = np.asarray(b, dtype=np.float32)

    np_stream = np.float16 if MODE == "f16" else np.float32

    # ---- host prep: sort by order, pad groups to 128, interleave in-group --
    counts = np.bincount(eo, minlength=NK)
    assert counts.size == NK, f"edge order out of range: {counts.size}"

    group_tiles = [(int(c) + P - 1) // P for c in counts]
    permA_parts = []     # A rows: padded sorted order (pad rows: garbage OK)
    permX_parts = []     # xet cols: interleaved within group
    valid_parts = []     # False where xet slot is padding
    for k in range(NK):
        idx = np.nonzero(eo == k)[0]
        tk = group_tiles[k]
        if tk == 0:
            continue
        gsz = tk * P
        src = np.zeros(gsz, dtype=np.int64)
        val = np.zeros(gsz, dtype=bool)
        src[:len(idx)] = idx
        val[:len(idx)] = True
        permA_parts.append(src)
        # interleave: final slot (j, p) (j = tile in group, p = partition)
        # takes sorted-group offset p*tk + j — matches the DMA access
        # pattern "(p o) n" that hands partition p rows p*tk + [j0, j0+nt)
        permX_parts.append(src.reshape(P, tk).T.reshape(-1))
        valid_parts.append(val.reshape(P, tk).T.reshape(-1))
    permA = np.concatenate(permA_parts)
    permX = np.concatenate(permX_parts)
    valid = np.concatenate(valid_parts)

    xe_pad = x_e[permX]
    xe_pad[~valid] = 0.0
    xet = np.ascontiguousarray(xe_pad.T.astype(np_stream))   # [64, e_pad]
    r = (1.0 / (1.0 + sn)).astype(np.float32)
    A = incidence.T[permA]                                   # [e_pad, N]
    A *= r[None, :]
    A = A.astype(np_stream)
    xvrt_full = np.ascontiguousarray((x_v * r[:, None]).T)   # [64, N]
    w0 = np.ascontiguousarray(W[0])
    w1 = np.ascontiguousarray(W[1]).astype(np_stream)
    w11 = np.ascontiguousarray(W[1, 1])
    bt = np.ascontiguousarray(b.reshape(1, D).T)             # [64, 1]

    # host-side reductions feeding the tiny x0 matmuls
    s6 = np.zeros((D, NK + 1), dtype=np.float32)
    for k in range(NK):
        if counts[k]:
            s6[:, k] = x_e[eo == k].sum(axis=0)
    s6[:, NK] = x_v.sum(axis=0)

    nc = _build_program(group_tiles)

    in_maps = []
    for m in range(NCORES):
        sl = slice(m * VS, (m + 1) * VS)
        in_maps.append({
            "xet": xet,
            "inct": np.ascontiguousarray(A[:, sl]),
            "xvrt": np.ascontiguousarray(xvrt_full[:, sl]),
            "w0": w0,
            "w1": w1,
            "w11": w11,
            "bt": bt,
            "s6": s6,
        })
    del A

    do_trace = TRACE and _ensure_ntff_hook()
    res = run_bass_kernel_spmd(nc, in_maps, core_ids=list(range(NCORES)),
                               trace=do_trace)
    LAST_EXEC_NS = res.exec_time_ns
    LAST_RESULTS = res

    out = np.empty((N, D), dtype=np.float32)
    for m in range(NCORES):
        out[m * VS:(m + 1) * VS, :] = res.results[m]["outt"].T
    return out
